# revision 2
# baseline (speedup 1.0000x reference)
"""DiceCE-with-ignore-index loss kernel for Trainium2, 8 NeuronCores.

Contract: kernel(logits, target) -> np.float32 scalar loss, matching
reference:  CE(mean over valid voxels) + masked soft Dice (batch dice,
background excluded), ignore_index = -1.

Strategy
--------
Data-parallel over (b, d): 2*64 = 128 (b,d) slices -> 16 per core.
Host casts logits/target to bf16 (values of target in {-1..3} are exact).
Per core the kernel reduces its 1,048,576 voxels to a handful of partial
sums; the tiny final combine happens on host in float64.

Per-voxel math (no max-subtraction needed: |x| <= ~6 for randn inputs):
  e_c = exp(x_c)            (ScalarE, bf16)
  s   = sum_c e_c           (VectorE adds)
  L   = log s               (ScalarE)
  r   = exp(-L) = 1/s       (ScalarE; ACT Reciprocal is banned)
  w   = (t>=0) * r          (fused scalar_tensor_tensor)
  z_c = e_c * w             (STT, free accum -> p_sum[c])
  eq_c = (t==c)             (tensor_scalar, free accum -> gt_sum[c])
  ca  = (t>=0) * L          (STT, free accum -> sum vf*L)
TensorE (PSUM-accumulated "diag trick"): for each class, stationary =
eq_c chunk, moving = [z_c | x_c] chunk; trace of the accumulated 128x128
blocks gives intersect[c] = sum eq_c*z_c and xb[c] = sum eq_c*x_c.
CE = (sum vf*L - sum_c xb[c]) / count.
"""
import os
import sys
from contextlib import ExitStack

for _p in ("/opt/trn_rl_repo", "/root/.axon_site/_ro/trn_rl_repo", "/root/.axon_site"):
    if os.path.isdir(_p) and _p not in sys.path:
        sys.path.append(_p)

import numpy as np
import ml_dtypes

import concourse.bass as bass
import concourse.tile as tile
from concourse import bacc, mybir
from concourse.bass_utils import run_bass_kernel_spmd

BF16 = mybir.dt.bfloat16
F32 = mybir.dt.float32
ALU = mybir.AluOpType
ACTF = mybir.ActivationFunctionType

P = 128          # partitions
FD = 2048        # free dim per megatile
NMT = 4          # megatiles per core (4 * 128 * 2048 = 1,048,576 voxels)
NCHUNK = FD // P # 16 diag chunks per megatile
NCORES = 8
C = 4            # classes

B, D, H, W = 2, 64, 256, 256
SMOOTH_NR = 1e-05
SMOOTH_DR = 1e-05

_NC_CACHE = {}


def _build_nc():
    nc = bacc.Bacc("TRN2", target_bir_lowering=False, debug=False)

    X = nc.dram_tensor("x", [C, NMT, P, FD], BF16, kind="ExternalInput")
    T = nc.dram_tensor("t", [NMT, P, FD], BF16, kind="ExternalInput")
    # acc columns per megatile: [ps1, ps2, ps3, gt0, gt1, gt2, gt3, ca]
    OUT_ACC = nc.dram_tensor("out_acc", [P, NMT * 8], F32, kind="ExternalOutput")
    # 3 classes x [z-block | x-block] (256 cols each) + class0 x-block (128)
    OUT_PS = nc.dram_tensor("out_ps", [P, 3 * 2 * P + P], F32, kind="ExternalOutput")

    with tile.TileContext(nc) as tc, ExitStack() as ctx:
        io = ctx.enter_context(tc.tile_pool(name="io", bufs=2))
        mid = ctx.enter_context(tc.tile_pool(name="mid", bufs=2))
        singles = ctx.enter_context(tc.tile_pool(name="singles", bufs=1))
        psum = ctx.enter_context(tc.tile_pool(name="psum", bufs=1, space="PSUM"))

        acc = singles.tile([P, NMT * 8], F32)
        ps = [psum.tile([P, 2 * P], F32, tag=f"ps{c}", name=f"ps{c}") for c in (1, 2, 3)]
        ps0 = psum.tile([P, P], F32)

        dma_engines = [nc.sync, nc.gpsimd]

        for mt in range(NMT):
            # ---- loads ----
            x0 = io.tile([P, FD], BF16, tag="x0")
            t_sb = io.tile([P, FD], BF16, tag="t")
            zx = [io.tile([P, 2, FD], BF16, tag=f"zx{c}", name=f"zx{c}_{mt}") for c in (1, 2, 3)]
            dma_engines[mt % 2].dma_start(x0[:], X[0, mt])
            dma_engines[(mt + 1) % 2].dma_start(t_sb[:], T[mt])
            for i, c in enumerate((1, 2, 3)):
                dma_engines[(mt + i) % 2].dma_start(zx[i][:, 1, :], X[c, mt])

            # ---- ScalarE: exponentials ----
            e = [mid.tile([P, FD], BF16, tag=f"e{c}", name=f"e{c}_{mt}") for c in range(C)]
            nc.scalar.activation(e[0][:], x0[:], ACTF.Exp)
            for i in range(3):
                nc.scalar.activation(e[i + 1][:], zx[i][:, 1, :], ACTF.Exp)

            # ---- VectorE: s = e0+e1+e2+e3 ----
            s01 = mid.tile([P, FD], BF16, tag="s01")
            s23 = mid.tile([P, FD], BF16, tag="s23")
            s = mid.tile([P, FD], BF16, tag="s")
            nc.vector.tensor_add(s01[:], e[0][:], e[1][:])
            nc.vector.tensor_add(s23[:], e[2][:], e[3][:])
            nc.vector.tensor_add(s[:], s01[:], s23[:])

            # ---- ScalarE: L = log s ; r = exp(-L) ----
            L = mid.tile([P, FD], BF16, tag="L")
            r = mid.tile([P, FD], BF16, tag="r")
            nc.scalar.activation(L[:], s[:], ACTF.Ln)
            nc.scalar.activation(r[:], L[:], ACTF.Exp, scale=-1.0)

            # ---- VectorE: fused mask products with free-dim accumulation ----
            w = mid.tile([P, FD], BF16, tag="w")
            nc.vector.scalar_tensor_tensor(
                out=w[:], in0=t_sb[:], scalar=0.0, in1=r[:],
                op0=ALU.is_ge, op1=ALU.mult)
            for i, c in enumerate((1, 2, 3)):
                nc.vector.scalar_tensor_tensor(
                    out=zx[i][:, 0, :], in0=e[c][:], scalar=1.0, in1=w[:],
                    op0=ALU.mult, op1=ALU.mult,
                    accum_out=acc[:, mt * 8 + i: mt * 8 + i + 1])
            eq = [mid.tile([P, FD], BF16, tag=f"eq{c}", name=f"eq{c}_{mt}") for c in range(C)]
            for c in range(C):
                nc.vector.tensor_scalar(
                    out=eq[c][:], in0=t_sb[:], scalar1=float(c), scalar2=None,
                    op0=ALU.is_equal, op1=ALU.add,
                    accum_out=acc[:, mt * 8 + 3 + c: mt * 8 + 4 + c])
            junk = singles.tile([P, FD], BF16)
            nc.vector.scalar_tensor_tensor(
                out=junk[:], in0=t_sb[:], scalar=0.0, in1=L[:],
                op0=ALU.is_ge, op1=ALU.mult,
                accum_out=acc[:, mt * 8 + 7: mt * 8 + 8])

            # ---- TensorE: diag-trick accumulation ----
            first = mt == 0
            last = mt == NMT - 1
            for k in range(NCHUNK):
                sl = slice(k * P, (k + 1) * P)
                for i in range(3):
                    nc.tensor.matmul(
                        ps[i][:], eq[i + 1][:, sl], zx[i][:, :, sl],
                        start=(first and k == 0), stop=(last and k == NCHUNK - 1))
                nc.tensor.matmul(
                    ps0[:], eq[0][:, sl], x0[:, sl],
                    start=(first and k == 0), stop=(last and k == NCHUNK - 1))

        # ---- epilogue: evacuate PSUM, write outputs ----
        ps_sb = singles.tile([P, 3 * 2 * P + P], F32)
        for i in range(3):
            nc.vector.tensor_copy(ps_sb[:, i * 2 * P:(i + 1) * 2 * P], ps[i][:])
        nc.vector.tensor_copy(ps_sb[:, 3 * 2 * P:], ps0[:])
        nc.sync.dma_start(OUT_ACC[:], acc[:])
        nc.sync.dma_start(OUT_PS[:], ps_sb[:])

    nc.compile()
    return nc


def _get_nc():
    if "nc" not in _NC_CACHE:
        _NC_CACHE["nc"] = _build_nc()
    return _NC_CACHE["nc"]


def _shard_inputs(logits: np.ndarray, target: np.ndarray):
    """Split into 8 per-core input maps; cast to bf16 on host."""
    assert logits.shape == (B, C, D, H, W), logits.shape
    assert target.shape == (B, 1, D, H, W), target.shape
    lg = np.ascontiguousarray(logits).astype(ml_dtypes.bfloat16)
    tg = target[:, 0].astype(np.float32).astype(ml_dtypes.bfloat16)

    d_per_core = D // (NCORES // B)  # 16
    in_maps = []
    for k in range(NCORES):
        b = k // (NCORES // B)
        d0 = (k % (NCORES // B)) * d_per_core
        xs = lg[b, :, d0:d0 + d_per_core].reshape(C, NMT, P, FD)
        ts = tg[b, d0:d0 + d_per_core].reshape(NMT, P, FD)
        in_maps.append({"x": np.ascontiguousarray(xs), "t": np.ascontiguousarray(ts)})
    return in_maps


def _combine(results) -> np.float32:
    ps_sum = np.zeros(3, np.float64)   # p_sum[c], c=1..3
    gt = np.zeros(C, np.float64)       # gt_sum[c], c=0..3
    ca = 0.0                           # sum vf * log s
    xb = np.zeros(C, np.float64)       # sum eq_c * x_c
    inter = np.zeros(3, np.float64)    # intersect[c], c=1..3

    for res in results:
        acc = res["out_acc"].astype(np.float64).reshape(P, NMT, 8)
        ps_sum += acc[:, :, 0:3].sum(axis=(0, 1))
        gt += acc[:, :, 3:7].sum(axis=(0, 1))
        ca += acc[:, :, 7].sum()
        blk = res["out_ps"].astype(np.float64)
        for i in range(3):
            inter[i] += np.trace(blk[:, i * 256: i * 256 + 128])
            xb[i + 1] += np.trace(blk[:, i * 256 + 128: i * 256 + 256])
        xb[0] += np.trace(blk[:, 768:896])

    count = gt.sum()
    ce = (ca - xb.sum()) / count

    gt_fg = gt[1:4]
    denom = ps_sum + gt_fg
    dice = (2.0 * inter + SMOOTH_NR) / (denom + SMOOTH_DR)
    present = (gt_fg > 0).astype(np.float64)
    n_present = present.sum()
    mean_dice = (dice * present).sum() / max(n_present, 1.0)
    dice_loss = (1.0 - mean_dice) if n_present > 0 else 0.0
    return np.float32(dice_loss + ce)


def kernel(logits: np.ndarray, target: np.ndarray) -> np.ndarray:
    nc = _get_nc()
    in_maps = _shard_inputs(np.asarray(logits), np.asarray(target))
    out = run_bass_kernel_spmd(nc, in_maps, core_ids=list(range(NCORES)))
    return _combine(out.results)


if __name__ == "__main__":
    rng = np.random.default_rng(0)
    lg = rng.standard_normal((B, C, D, H, W), dtype=np.float32)
    tg = rng.integers(-1, C, (B, 1, D, H, W)).astype(np.int32)
    print(kernel(lg, tg))


# revision 3
# speedup vs baseline: 1.1752x; 1.1752x over previous
"""DiceCE-with-ignore-index loss kernel for Trainium2, 8 NeuronCores.

Contract: kernel(logits, target) -> np.float32 scalar loss, matching
reference: CE (mean over valid voxels) + masked soft Dice (batch dice,
background excluded), ignore_index = -1.

Strategy
--------
Data-parallel over (b, d): 2*64 = 128 (b,d) slices -> 16 per core.
Host casts logits/target to bf16 (target values {-1..3} are exact).
Per core the kernel reduces its 1,048,576 voxels to partial sums; the
tiny (C,)-vector final combine happens on host in float64.

Per-voxel math (no max-subtraction: |x| <= ~6 for randn inputs):
  e_c = exp(x_c)                       ScalarE (one table load: Exp+Ln
  L   = log s,  r = exp(-L) = 1/s      forced into the combined set)
  s   = sum_c e_c                      VectorE adds (bf16 2x mode)
  vf  = (t>=0); w = vf*r               VectorE TS(4x) + TT(2x)
  z_c = e_c*w  (+free accum->p_sum[c]) VectorE scalar_tensor_tensor
  eq_c = (t==c)                        VectorE TS (4x)
  ca  = (t>=0)*L (+accum)              VectorE scalar_tensor_tensor
TensorE computes the masked sums via PSUM-accumulated "diag trick"
matmuls: stationary = eq_c chunk, moving = packed [z_c | x_c | ones]
planes; trace(block0) = intersect[c], trace(block1) = sum eq_c*x_c,
block2 column = gt_sum[c].  Class 0 (needed for CE only) uses
[x_0 | ones].  CE = (sum vf*L - sum_c sum eq_c*x_c) / count.
"""
import os
import sys
from contextlib import ExitStack

for _p in ("/opt/trn_rl_repo", "/root/.axon_site/_ro/trn_rl_repo", "/root/.axon_site"):
    if os.path.isdir(_p) and _p not in sys.path:
        sys.path.append(_p)

import numpy as np
import ml_dtypes

import concourse.bass as bass
import concourse.tile as tile
from concourse import bacc, mybir
from concourse.bass_utils import run_bass_kernel_spmd

BF16 = mybir.dt.bfloat16
F32 = mybir.dt.float32
ALU = mybir.AluOpType
ACTF = mybir.ActivationFunctionType

P = 128          # partitions
FD = 2048        # free dim per megatile
NMT = 4          # megatiles per core (4 * 128 * 2048 = 1,048,576 voxels)
NCHUNK = FD // P # 16 diag chunks per megatile
NCORES = 8
C = 4            # classes

B, D, H, W = 2, 64, 256, 256
SMOOTH_NR = 1e-05
SMOOTH_DR = 1e-05

_NC_CACHE = {}


def _patch_act_tables():
    """Force Exp and Ln to resolve to the combined natural_log_exp set so the
    kernel needs a single ACT_TABLE_LOAD instead of thrashing between the
    exp-only and ln-only sets every megatile."""
    import concourse.hw_specs as hw_specs
    if getattr(bacc, "_act_tables_patched", False):
        return
    orig = hw_specs.get_activation_tables

    def patched(arch):
        tables = {k: set(v) for k, v in orig(arch).items()}
        if "natural_log_exp_and_others" in tables:
            for name, fns in tables.items():
                if name != "natural_log_exp_and_others":
                    fns.discard(ACTF.Exp)
                    fns.discard(ACTF.Ln)
        return tables

    hw_specs.get_activation_tables = patched
    bacc.get_activation_tables = patched
    bacc._act_tables_patched = True


def _build_nc():
    _patch_act_tables()
    nc = bacc.Bacc("TRN2", target_bir_lowering=False, debug=False)

    X = nc.dram_tensor("x", [C, NMT, P, FD], BF16, kind="ExternalInput")
    T = nc.dram_tensor("t", [NMT, P, FD], BF16, kind="ExternalInput")
    # acc columns per megatile: [ps1, ps2, ps3, ca]
    OUT_ACC = nc.dram_tensor("out_acc", [P, NMT * 4], F32, kind="ExternalOutput")
    # per class c=1..3: [z-diag 128 | x-diag 128 | ones-cols 128] = 384
    # class 0: [x0-diag 128 | ones-cols 128] = 256
    OUT_PS = nc.dram_tensor("out_ps", [P, 3 * 384 + 256], F32, kind="ExternalOutput")

    with tile.TileContext(nc) as tc, ExitStack() as ctx:
        io = ctx.enter_context(tc.tile_pool(name="io", bufs=2))
        mid = ctx.enter_context(tc.tile_pool(name="mid", bufs=2))
        one = ctx.enter_context(tc.tile_pool(name="one", bufs=1))
        psum = ctx.enter_context(tc.tile_pool(name="psum", bufs=1, space="PSUM"))

        acc = one.tile([P, NMT * 4], F32)
        ps = [psum.tile([P, 384], F32, name=f"ps{c}") for c in (1, 2, 3)]
        ps0 = psum.tile([P, 256], F32)

        # manual double buffers with a persistent ones-plane (plane 2 / 1)
        zxbuf = [[one.tile([P, 3, FD], BF16, name=f"zx{c}_{ab}") for c in (1, 2, 3)]
                 for ab in range(2)]
        x0buf = [one.tile([P, 2, FD], BF16, name=f"x0_{ab}") for ab in range(2)]
        for ab in range(2):
            for z in zxbuf[ab]:
                nc.vector.memset(z[:, 2, :], 1.0)
            nc.vector.memset(x0buf[ab][:, 1, :], 1.0)

        dma_engines = [nc.sync, nc.gpsimd]

        for mt in range(NMT):
            zx = zxbuf[mt % 2]
            x0 = x0buf[mt % 2]
            t_sb = io.tile([P, FD], BF16, tag="t", name=f"t_{mt}")
            dma_engines[mt % 2].dma_start(x0[:, 0, :], X[0, mt])
            dma_engines[(mt + 1) % 2].dma_start(t_sb[:], T[mt])
            for i, c in enumerate((1, 2, 3)):
                dma_engines[(mt + i) % 2].dma_start(zx[i][:, 1, :], X[c, mt])

            # ---- ScalarE ----
            e = [mid.tile([P, FD], BF16, tag=f"e{c}", name=f"e{c}_{mt}") for c in range(C)]
            nc.scalar.activation(e[0][:], x0[:, 0, :], ACTF.Exp)
            for i in range(3):
                nc.scalar.activation(e[i + 1][:], zx[i][:, 1, :], ACTF.Exp)

            # ---- VectorE: s = e0+e1+e2+e3 (bf16 TT = 2x mode) ----
            s01 = one.tile([P, FD], BF16, name=f"s01")
            s23 = one.tile([P, FD], BF16, name=f"s23")
            s = one.tile([P, FD], BF16, name=f"s")
            nc.vector.tensor_add(s01[:], e[0][:], e[1][:])
            nc.vector.tensor_add(s23[:], e[2][:], e[3][:])
            nc.vector.tensor_add(s[:], s01[:], s23[:])

            # ---- ScalarE: L = log s ; r = exp(-L) ----
            L = mid.tile([P, FD], BF16, tag="L", name=f"L_{mt}")
            r = mid.tile([P, FD], BF16, tag="r", name=f"r_{mt}")
            nc.scalar.activation(L[:], s[:], ACTF.Ln)
            nc.scalar.activation(r[:], L[:], ACTF.Exp, scale=-1.0)

            # ---- VectorE ----
            vf = one.tile([P, FD], BF16, name="vf")
            nc.vector.tensor_scalar(out=vf[:], in0=t_sb[:], scalar1=0.0, scalar2=None,
                                    op0=ALU.is_ge)
            w = one.tile([P, FD], BF16, name="w")
            nc.vector.tensor_mul(w[:], vf[:], r[:])
            for i, c in enumerate((1, 2, 3)):
                nc.vector.scalar_tensor_tensor(
                    out=zx[i][:, 0, :], in0=e[c][:], scalar=1.0, in1=w[:],
                    op0=ALU.mult, op1=ALU.mult,
                    accum_out=acc[:, mt * 4 + i: mt * 4 + i + 1])
            eq = [mid.tile([P, FD], BF16, tag=f"eq{c}", name=f"eq{c}_{mt}") for c in range(C)]
            for c in range(C):
                nc.vector.tensor_scalar(
                    out=eq[c][:], in0=t_sb[:], scalar1=float(c), scalar2=None,
                    op0=ALU.is_equal)
            nc.vector.scalar_tensor_tensor(
                out=s23[:], in0=t_sb[:], scalar=0.0, in1=L[:],
                op0=ALU.is_ge, op1=ALU.mult,
                accum_out=acc[:, mt * 4 + 3: mt * 4 + 4])

            # ---- TensorE: diag-trick accumulation ----
            first = mt == 0
            last = mt == NMT - 1
            for k in range(NCHUNK):
                sl = slice(k * P, (k + 1) * P)
                for i in range(3):
                    nc.tensor.matmul(
                        ps[i][:], eq[i + 1][:, sl], zx[i][:, :, sl],
                        start=(first and k == 0), stop=(last and k == NCHUNK - 1))
                nc.tensor.matmul(
                    ps0[:], eq[0][:, sl], x0[:, :, sl],
                    start=(first and k == 0), stop=(last and k == NCHUNK - 1))

        # ---- epilogue ----
        ps_sb = one.tile([P, 3 * 384 + 256], F32)
        for i in range(3):
            nc.vector.tensor_copy(ps_sb[:, i * 384:(i + 1) * 384], ps[i][:])
        nc.vector.tensor_copy(ps_sb[:, 3 * 384:], ps0[:])
        nc.sync.dma_start(OUT_ACC[:], acc[:])
        nc.sync.dma_start(OUT_PS[:], ps_sb[:])

    nc.compile()
    return nc


def _get_nc():
    if "nc" not in _NC_CACHE:
        _NC_CACHE["nc"] = _build_nc()
    return _NC_CACHE["nc"]


def _shard_inputs(logits: np.ndarray, target: np.ndarray):
    """Split into 8 per-core input maps; cast to bf16 on host."""
    assert logits.shape == (B, C, D, H, W), logits.shape
    assert target.shape == (B, 1, D, H, W), target.shape
    lg = np.ascontiguousarray(logits).astype(ml_dtypes.bfloat16)
    tg = target[:, 0].astype(np.float32).astype(ml_dtypes.bfloat16)

    d_per_core = D // (NCORES // B)  # 16
    in_maps = []
    for k in range(NCORES):
        b = k // (NCORES // B)
        d0 = (k % (NCORES // B)) * d_per_core
        xs = lg[b, :, d0:d0 + d_per_core].reshape(C, NMT, P, FD)
        ts = tg[b, d0:d0 + d_per_core].reshape(NMT, P, FD)
        in_maps.append({"x": np.ascontiguousarray(xs), "t": np.ascontiguousarray(ts)})
    return in_maps


def _combine(results) -> np.float32:
    ps_sum = np.zeros(3, np.float64)   # p_sum[c], c=1..3
    gt = np.zeros(C, np.float64)       # gt_sum[c], c=0..3
    ca = 0.0                           # sum vf * log s
    xb = np.zeros(C, np.float64)       # sum eq_c * x_c
    inter = np.zeros(3, np.float64)    # intersect[c], c=1..3

    for res in results:
        acc = res["out_acc"].astype(np.float64).reshape(P, NMT, 4)
        ps_sum += acc[:, :, 0:3].sum(axis=(0, 1))
        ca += acc[:, :, 3].sum()
        blk = res["out_ps"].astype(np.float64)
        for i in range(3):
            b0 = i * 384
            inter[i] += np.trace(blk[:, b0:b0 + 128])
            xb[i + 1] += np.trace(blk[:, b0 + 128:b0 + 256])
            gt[i + 1] += blk[:, b0 + 256].sum()
        xb[0] += np.trace(blk[:, 1152:1280])
        gt[0] += blk[:, 1280].sum()

    count = gt.sum()
    ce = (ca - xb.sum()) / count

    gt_fg = gt[1:4]
    denom = ps_sum + gt_fg
    dice = (2.0 * inter + SMOOTH_NR) / (denom + SMOOTH_DR)
    present = (gt_fg > 0).astype(np.float64)
    n_present = present.sum()
    mean_dice = (dice * present).sum() / max(n_present, 1.0)
    dice_loss = (1.0 - mean_dice) if n_present > 0 else 0.0
    return np.float32(dice_loss + ce)


def kernel(logits: np.ndarray, target: np.ndarray) -> np.ndarray:
    nc = _get_nc()
    in_maps = _shard_inputs(np.asarray(logits), np.asarray(target))
    out = run_bass_kernel_spmd(nc, in_maps, core_ids=list(range(NCORES)))
    return _combine(out.results)


if __name__ == "__main__":
    rng = np.random.default_rng(0)
    lg = rng.standard_normal((B, C, D, H, W), dtype=np.float32)
    tg = rng.integers(-1, C, (B, 1, D, H, W)).astype(np.int32)
    print(kernel(lg, tg))


# revision 4
# speedup vs baseline: 1.1843x; 1.0077x over previous
"""DiceCE-with-ignore-index loss kernel for Trainium2, 8 NeuronCores.

Contract: kernel(logits, target) -> np.float32 scalar loss, matching
reference: CE (mean over valid voxels) + masked soft Dice (batch dice,
background excluded), ignore_index = -1.

Strategy
--------
Data-parallel over (b, d): 2*64 = 128 (b,d) slices -> 16 per core.
Host casts logits/target to bf16 (target values {-1..3} are exact).
Per core the kernel reduces its 1,048,576 voxels to partial sums; the
tiny (C,)-vector final combine happens on host in float64.

Per-voxel math (no max-subtraction: |x| <= ~6 for randn inputs):
  e_c = exp(x_c)                       ScalarE (one table load: Exp+Ln
  L   = log s,  r = exp(-L) = 1/s      forced into the combined set)
  s   = sum_c e_c                      VectorE adds (bf16 2x mode)
  vf  = (t>=0); w = vf*r               VectorE TS(4x) + TT(2x)
  z_c = e_c*w  (+free accum->p_sum[c]) VectorE scalar_tensor_tensor
  eq_c = (t==c)                        VectorE TS (4x)
  ca  = (t>=0)*L (+accum)              VectorE scalar_tensor_tensor
TensorE computes the masked sums via PSUM-accumulated "diag trick"
matmuls: stationary = eq_c chunk, moving = packed [z_c | x_c | ones]
planes; trace(block0) = intersect[c], trace(block1) = sum eq_c*x_c,
block2 column = gt_sum[c].  Class 0 (needed for CE only) uses
[x_0 | ones].  CE = (sum vf*L - sum_c sum eq_c*x_c) / count.
"""
import os
import sys
from contextlib import ExitStack

for _p in ("/opt/trn_rl_repo", "/root/.axon_site/_ro/trn_rl_repo", "/root/.axon_site"):
    if os.path.isdir(_p) and _p not in sys.path:
        sys.path.append(_p)

import numpy as np
import ml_dtypes

import concourse.bass as bass
import concourse.tile as tile
from concourse import bacc, mybir
from concourse.bass_utils import run_bass_kernel_spmd

BF16 = mybir.dt.bfloat16
F32 = mybir.dt.float32
ALU = mybir.AluOpType
ACTF = mybir.ActivationFunctionType

P = 128          # partitions
FD = 1024        # free dim per megatile (small so PE idle gaps stay under
                 # the ~3.4us HAM window and TensorE holds its 2.4 GHz clock)
NMT = 8          # megatiles per core (8 * 128 * 1024 = 1,048,576 voxels)
NCHUNK = FD // P # 16 diag chunks per megatile
NCORES = 8
C = 4            # classes

B, D, H, W = 2, 64, 256, 256
SMOOTH_NR = 1e-05
SMOOTH_DR = 1e-05

_NC_CACHE = {}


def _patch_act_tables():
    """Force Exp and Ln to resolve to the combined natural_log_exp set so the
    kernel needs a single ACT_TABLE_LOAD instead of thrashing between the
    exp-only and ln-only sets every megatile."""
    import concourse.hw_specs as hw_specs
    if getattr(bacc, "_act_tables_patched", False):
        return
    orig = hw_specs.get_activation_tables

    def patched(arch):
        tables = {k: set(v) for k, v in orig(arch).items()}
        if "natural_log_exp_and_others" in tables:
            for name, fns in tables.items():
                if name != "natural_log_exp_and_others":
                    fns.discard(ACTF.Exp)
                    fns.discard(ACTF.Ln)
        return tables

    hw_specs.get_activation_tables = patched
    bacc.get_activation_tables = patched
    bacc._act_tables_patched = True


def _build_nc():
    _patch_act_tables()
    nc = bacc.Bacc("TRN2", target_bir_lowering=False, debug=False)

    X = nc.dram_tensor("x", [C, NMT, P, FD], BF16, kind="ExternalInput")
    T = nc.dram_tensor("t", [NMT, P, FD], BF16, kind="ExternalInput")
    # acc columns per megatile: [ps1, ps2, ps3, ca]
    OUT_ACC = nc.dram_tensor("out_acc", [P, NMT * 4], F32, kind="ExternalOutput")
    # per class c=1..3: [z-diag 128 | x-diag 128 | ones-cols 128] = 384
    # class 0: [x0-diag 128 | ones-cols 128] = 256
    OUT_PS = nc.dram_tensor("out_ps", [P, 3 * 384 + 256], F32, kind="ExternalOutput")

    with tile.TileContext(nc) as tc, ExitStack() as ctx:
        io = ctx.enter_context(tc.tile_pool(name="io", bufs=2))
        mid = ctx.enter_context(tc.tile_pool(name="mid", bufs=2))
        one = ctx.enter_context(tc.tile_pool(name="one", bufs=1))
        psum = ctx.enter_context(tc.tile_pool(name="psum", bufs=1, space="PSUM"))

        acc = one.tile([P, NMT * 4], F32)
        ps = [psum.tile([P, 384], F32, name=f"ps{c}") for c in (1, 2, 3)]
        ps0 = psum.tile([P, 256], F32)

        # manual double buffers with a persistent ones-plane (plane 2 / 1)
        zxbuf = [[one.tile([P, 3, FD], BF16, name=f"zx{c}_{ab}") for c in (1, 2, 3)]
                 for ab in range(2)]
        x0buf = [one.tile([P, 2, FD], BF16, name=f"x0_{ab}") for ab in range(2)]
        for ab in range(2):
            for z in zxbuf[ab]:
                nc.gpsimd.memset(z[:, 2, :], 1.0)
            nc.gpsimd.memset(x0buf[ab][:, 1, :], 1.0)

        dma_engines = [nc.sync, nc.gpsimd]

        for mt in range(NMT):
            zx = zxbuf[mt % 2]
            x0 = x0buf[mt % 2]
            t_sb = io.tile([P, FD], BF16, tag="t", name=f"t_{mt}")
            dma_engines[mt % 2].dma_start(x0[:, 0, :], X[0, mt])
            dma_engines[(mt + 1) % 2].dma_start(t_sb[:], T[mt])
            for i, c in enumerate((1, 2, 3)):
                dma_engines[(mt + i) % 2].dma_start(zx[i][:, 1, :], X[c, mt])

            # ---- ScalarE ----
            e = [mid.tile([P, FD], BF16, tag=f"e{c}", name=f"e{c}_{mt}") for c in range(C)]
            nc.scalar.activation(e[0][:], x0[:, 0, :], ACTF.Exp)
            for i in range(3):
                nc.scalar.activation(e[i + 1][:], zx[i][:, 1, :], ACTF.Exp)

            # ---- VectorE: s = e0+e1+e2+e3 (bf16 TT = 2x mode) ----
            s01 = one.tile([P, FD], BF16, name=f"s01")
            s23 = one.tile([P, FD], BF16, name=f"s23")
            s = one.tile([P, FD], BF16, name=f"s")
            nc.vector.tensor_add(s01[:], e[0][:], e[1][:])
            nc.vector.tensor_add(s23[:], e[2][:], e[3][:])
            nc.vector.tensor_add(s[:], s01[:], s23[:])

            # ---- ScalarE: L = log s ; r = exp(-L) ----
            L = mid.tile([P, FD], BF16, tag="L", name=f"L_{mt}")
            r = mid.tile([P, FD], BF16, tag="r", name=f"r_{mt}")
            nc.scalar.activation(L[:], s[:], ACTF.Ln)
            nc.scalar.activation(r[:], L[:], ACTF.Exp, scale=-1.0)

            # ---- VectorE ----
            vf = one.tile([P, FD], BF16, name="vf")
            nc.vector.tensor_scalar(out=vf[:], in0=t_sb[:], scalar1=0.0, scalar2=None,
                                    op0=ALU.is_ge)
            w = one.tile([P, FD], BF16, name="w")
            nc.vector.tensor_mul(w[:], vf[:], r[:])
            for i, c in enumerate((1, 2, 3)):
                nc.vector.scalar_tensor_tensor(
                    out=zx[i][:, 0, :], in0=e[c][:], scalar=1.0, in1=w[:],
                    op0=ALU.mult, op1=ALU.mult,
                    accum_out=acc[:, mt * 4 + i: mt * 4 + i + 1])
            eq = [mid.tile([P, FD], BF16, tag=f"eq{c}", name=f"eq{c}_{mt}") for c in range(C)]
            for c in range(C):
                nc.vector.tensor_scalar(
                    out=eq[c][:], in0=t_sb[:], scalar1=float(c), scalar2=None,
                    op0=ALU.is_equal)
            nc.vector.scalar_tensor_tensor(
                out=s23[:], in0=t_sb[:], scalar=0.0, in1=L[:],
                op0=ALU.is_ge, op1=ALU.mult,
                accum_out=acc[:, mt * 4 + 3: mt * 4 + 4])

            # ---- TensorE: diag-trick accumulation ----
            first = mt == 0
            last = mt == NMT - 1
            for k in range(NCHUNK):
                sl = slice(k * P, (k + 1) * P)
                for i in range(3):
                    nc.tensor.matmul(
                        ps[i][:], eq[i + 1][:, sl], zx[i][:, :, sl],
                        start=(first and k == 0), stop=(last and k == NCHUNK - 1))
                nc.tensor.matmul(
                    ps0[:], eq[0][:, sl], x0[:, :, sl],
                    start=(first and k == 0), stop=(last and k == NCHUNK - 1))

        # ---- epilogue ----
        ps_sb = one.tile([P, 3 * 384 + 256], F32)
        for i in range(3):
            nc.vector.tensor_copy(ps_sb[:, i * 384:(i + 1) * 384], ps[i][:])
        nc.vector.tensor_copy(ps_sb[:, 3 * 384:], ps0[:])
        nc.sync.dma_start(OUT_ACC[:], acc[:])
        nc.sync.dma_start(OUT_PS[:], ps_sb[:])

    nc.compile()
    return nc


def _get_nc():
    if "nc" not in _NC_CACHE:
        _NC_CACHE["nc"] = _build_nc()
    return _NC_CACHE["nc"]


def _shard_inputs(logits: np.ndarray, target: np.ndarray):
    """Split into 8 per-core input maps; cast to bf16 on host."""
    assert logits.shape == (B, C, D, H, W), logits.shape
    assert target.shape == (B, 1, D, H, W), target.shape
    lg = np.ascontiguousarray(logits).astype(ml_dtypes.bfloat16)
    tg = target[:, 0].astype(np.float32).astype(ml_dtypes.bfloat16)

    d_per_core = D // (NCORES // B)  # 16
    in_maps = []
    for k in range(NCORES):
        b = k // (NCORES // B)
        d0 = (k % (NCORES // B)) * d_per_core
        xs = lg[b, :, d0:d0 + d_per_core].reshape(C, NMT, P, FD)
        ts = tg[b, d0:d0 + d_per_core].reshape(NMT, P, FD)
        in_maps.append({"x": np.ascontiguousarray(xs), "t": np.ascontiguousarray(ts)})
    return in_maps


def _combine(results) -> np.float32:
    ps_sum = np.zeros(3, np.float64)   # p_sum[c], c=1..3
    gt = np.zeros(C, np.float64)       # gt_sum[c], c=0..3
    ca = 0.0                           # sum vf * log s
    xb = np.zeros(C, np.float64)       # sum eq_c * x_c
    inter = np.zeros(3, np.float64)    # intersect[c], c=1..3

    for res in results:
        acc = res["out_acc"].astype(np.float64).reshape(P, NMT, 4)
        ps_sum += acc[:, :, 0:3].sum(axis=(0, 1))
        ca += acc[:, :, 3].sum()
        blk = res["out_ps"].astype(np.float64)
        for i in range(3):
            b0 = i * 384
            inter[i] += np.trace(blk[:, b0:b0 + 128])
            xb[i + 1] += np.trace(blk[:, b0 + 128:b0 + 256])
            gt[i + 1] += blk[:, b0 + 256].sum()
        xb[0] += np.trace(blk[:, 1152:1280])
        gt[0] += blk[:, 1280].sum()

    count = gt.sum()
    ce = (ca - xb.sum()) / count

    gt_fg = gt[1:4]
    denom = ps_sum + gt_fg
    dice = (2.0 * inter + SMOOTH_NR) / (denom + SMOOTH_DR)
    present = (gt_fg > 0).astype(np.float64)
    n_present = present.sum()
    mean_dice = (dice * present).sum() / max(n_present, 1.0)
    dice_loss = (1.0 - mean_dice) if n_present > 0 else 0.0
    return np.float32(dice_loss + ce)


def kernel(logits: np.ndarray, target: np.ndarray) -> np.ndarray:
    nc = _get_nc()
    in_maps = _shard_inputs(np.asarray(logits), np.asarray(target))
    out = run_bass_kernel_spmd(nc, in_maps, core_ids=list(range(NCORES)))
    return _combine(out.results)


if __name__ == "__main__":
    rng = np.random.default_rng(0)
    lg = rng.standard_normal((B, C, D, H, W), dtype=np.float32)
    tg = rng.integers(-1, C, (B, 1, D, H, W)).astype(np.int32)
    print(kernel(lg, tg))
